# revision 19
# baseline (speedup 1.0000x reference)
"""ClusterScaleBiasBlock Trainium2 kernel.

Computes out = BN(x) * (1 + Wg[ids]) + Wb[ids] for
x:[32768,2048] f32, Wg/Wb:[64,2048], ids:[32768] int32, where
BN(x) = (x - mean) * rsqrt(var+eps) * gamma + beta (inference mode).

Algebraic folding (host side, tiny [64,2048] tables):
    inv  = rsqrt(var + eps) * gamma
    S[c] = inv * (1 + Wg[c])
    T[c] = (beta - mean*inv) * (1 + Wg[c]) + Wb[c]
    out  = x * S[ids] + T[ids]

Device side (8 cores, data-parallel over batch, 4096 rows each):
  - The per-row scale/bias gather S[ids]/T[ids] runs on TensorE as a
    one-hot matmul in bf16 with an exact hi/lo split: rhs stacks
    [S_hi; S_lo] ([128, F], K=128) and lhsT stacks the one-hot twice,
    so one matmul accumulates gather(S_hi)+gather(S_lo) in fp32 PSUM
    (~2^-17 relative error; bf16 one-hot rows are exact).
  - out = x*s + t as two VectorE tensor_tensor ops (fp32).
  - x/out move in 2 MB DMA transfers ([128, 4096] tiles: partition p
    holds two consecutive batch rows).  HBM traffic is just x in +
    out, so the kernel is DMA-bound.
"""

import sys

if "/opt/trn_rl_repo" not in sys.path:
    sys.path.insert(0, "/opt/trn_rl_repo")

import numpy as np

B, F, C = 32768, 2048, 64
N_CORES = 8
RPC = B // N_CORES  # rows per core = 4096
P = 128             # partition tile height
BN_EPS = 1e-3

_PROGRAM = None


def _build_program(rows=RPC):
    import concourse.bass as bass
    import concourse.bacc as bacc
    import concourse.mybir as mybir
    from concourse import tile

    f32 = mybir.dt.float32
    bf16 = mybir.dt.bfloat16
    nc = bacc.Bacc(None)
    n_dt = rows // (2 * P)        # DMA tiles, 256 rows each
    # x / out viewed as [rows/2, 2F]: row r' = batch rows (2r', 2r'+1).
    x_d = nc.declare_dram_parameter("x", [rows // 2, 2 * F], f32, isOutput=False)
    # [S_hi;S_lo] | [T_hi;T_lo] | one-hot (stacked twice), all bf16.
    # One DMA -> one wait for PE.  OH column block b=2i+h holds the
    # one-hot for (DMA tile i, half h): col p = batch row 256i+2p+h.
    tabs_d = nc.declare_dram_parameter("tabs", [2 * C, 2 * F + 2 * P * n_dt],
                                       bf16, isOutput=False)
    out_d = nc.declare_dram_parameter("out", [rows // 2, 2 * F], f32, isOutput=True)

    NC_ = 512  # psum bank limit: fp32 out, 512 per matmul
    with tile.TileContext(nc) as tc:
        with (
            tc.tile_pool(name="const", bufs=1) as cpool,
            tc.tile_pool(name="xin", bufs=2) as xpool,
            tc.tile_pool(name="oout", bufs=3) as opool,
            tc.tile_pool(name="mid", bufs=2) as mpool,
            tc.tile_pool(name="ps", bufs=1, space=bass.MemorySpace.PSUM) as pspool,
        ):
            tabs_sb = cpool.tile([2 * C, 2 * F + 2 * P * n_dt], bf16, tag="tabs")
            nc.sync.dma_start(out=tabs_sb[:], in_=tabs_d[:])
            Shl = tabs_sb[:, 0:F]
            Thl = tabs_sb[:, F:2 * F]
            OH_sb = tabs_sb[:, 2 * F:]

            for i in range(n_dt):
                xt = xpool.tile([P, 2 * F], f32, tag="x")
                if i == 0:
                    # split the first load so compute starts after 1 MB
                    nc.sync.dma_start(out=xt[:, 0:F], in_=x_d[0:P, 0:F])
                    nc.sync.dma_start(out=xt[:, F:], in_=x_d[0:P, F:])
                else:
                    nc.sync.dma_start(out=xt[:], in_=x_d[i * P:(i + 1) * P, :])
                ot = opool.tile([P, 2 * F], f32, tag="o")

                for h in range(2):
                    s_ps = pspool.tile([P, F], f32, tag="s")
                    t_ps = pspool.tile([P, F], f32, tag="t")
                    b = 2 * i + h
                    lhsT = OH_sb[:, b * P:(b + 1) * P]
                    for j in range(F // NC_):
                        cs = slice(j * NC_, (j + 1) * NC_)
                        nc.tensor.matmul(s_ps[:, cs], lhsT, Shl[:, cs],
                                         start=True, stop=True)
                        nc.tensor.matmul(t_ps[:, cs], lhsT, Thl[:, cs],
                                         start=True, stop=True)

                    hs = slice(h * F, (h + 1) * F)
                    mt = mpool.tile([P, F], f32, tag="m")
                    nc.vector.tensor_mul(mt[:], xt[:, hs], s_ps[:])
                    nc.vector.tensor_add(ot[:, hs], mt[:], t_ps[:])

                # stores ride the second HWDGE ring (ACT) so they don't
                # queue behind x loads on the SP ring
                if i == n_dt - 1:
                    # split the last store so the tail drains sooner
                    nc.scalar.dma_start(out=out_d[i * P:(i + 1) * P, 0:F],
                                        in_=ot[:, 0:F])
                    nc.scalar.dma_start(out=out_d[i * P:(i + 1) * P, F:],
                                        in_=ot[:, F:])
                else:
                    nc.scalar.dma_start(out=out_d[i * P:(i + 1) * P, :], in_=ot[:])
    nc.compile()
    return nc


def _host_tables(Wg, Wb, bn_gamma, bn_beta, moving_mean, moving_var):
    inv = (bn_gamma.astype(np.float64)
           / np.sqrt(moving_var.astype(np.float64) + BN_EPS))
    gp1 = 1.0 + Wg.astype(np.float64)  # [C, F]
    S = (inv[None, :] * gp1).astype(np.float32)
    T = ((bn_beta.astype(np.float64) - moving_mean.astype(np.float64) * inv)[None, :]
         * gp1 + Wb.astype(np.float64)).astype(np.float32)
    return S, T


def _pack_tabs(S, T, ids_c):
    """Build the per-core [2C, 2F + rows] bf16 constant block."""
    import ml_dtypes

    bf16 = ml_dtypes.bfloat16
    S_hi = S.astype(bf16)
    S_lo = (S - S_hi.astype(np.float32)).astype(bf16)
    T_hi = T.astype(bf16)
    T_lo = (T - T_hi.astype(np.float32)).astype(bf16)
    Shl = np.concatenate([S_hi, S_lo], axis=0)  # [2C, F]
    Thl = np.concatenate([T_hi, T_lo], axis=0)

    rows = ids_c.shape[0]
    n_dt = rows // (2 * P)
    # ids by (tile, partition, half): batch row 256i + 2p + h
    ids_r = ids_c.reshape(n_dt, P, 2)           # [i, p, h]
    oh = np.zeros((C, n_dt, 2, P), np.float32)
    i_ix, p_ix, h_ix = np.meshgrid(np.arange(n_dt), np.arange(P), np.arange(2),
                                   indexing="ij")
    oh[ids_r[i_ix, p_ix, h_ix], i_ix, h_ix, p_ix] = 1.0
    oh = oh.reshape(C, n_dt * 2 * P).astype(bf16)   # col block b=2i+h
    oh2 = np.concatenate([oh, oh], axis=0)          # stacked for K=2C
    return np.ascontiguousarray(np.concatenate([Shl, Thl, oh2], axis=1))


LAST_RESULT = None


def kernel(x, Wg, Wb, bn_gamma, bn_beta, moving_mean, moving_var, cluster_ids):
    global _PROGRAM, LAST_RESULT
    from concourse.bass_utils import run_bass_kernel_spmd

    x = np.ascontiguousarray(np.asarray(x, dtype=np.float32))
    ids = np.asarray(cluster_ids, dtype=np.int32)
    S, T = _host_tables(
        np.asarray(Wg, np.float32), np.asarray(Wb, np.float32),
        np.asarray(bn_gamma, np.float32), np.asarray(bn_beta, np.float32),
        np.asarray(moving_mean, np.float32), np.asarray(moving_var, np.float32),
    )

    in_maps = []
    for c in range(N_CORES):
        ids_c = ids[c * RPC:(c + 1) * RPC]
        in_maps.append({
            "x": x[c * RPC:(c + 1) * RPC].reshape(RPC // 2, 2 * F),
            "tabs": _pack_tabs(S, T, ids_c),
        })

    if _PROGRAM is None:
        _PROGRAM = _build_program()

    res = run_bass_kernel_spmd(_PROGRAM, in_maps, list(range(N_CORES)))
    LAST_RESULT = res
    out = np.concatenate(
        [r["out"].reshape(RPC, F) for r in res.results], axis=0)
    return out


if __name__ == "__main__":
    # Smoke test with random data against a local numpy reference.
    rng = np.random.default_rng(0)
    inputs = {
        "x": rng.standard_normal((B, F), dtype=np.float32),
        "Wg": 0.25 * rng.standard_normal((C, F)).astype(np.float32),
        "Wb": 0.25 * rng.standard_normal((C, F)).astype(np.float32),
        "bn_gamma": np.ones(F, np.float32),
        "bn_beta": np.zeros(F, np.float32),
        "moving_mean": 0.1 * rng.standard_normal(F).astype(np.float32),
        "moving_var": rng.uniform(0.5, 1.5, F).astype(np.float32),
        "cluster_ids": rng.integers(0, C, B, dtype=np.int32),
    }
    out = kernel(**inputs)
    inv = inputs["bn_gamma"] / np.sqrt(inputs["moving_var"] + BN_EPS)
    xn = (inputs["x"] - inputs["moving_mean"]) * inv + inputs["bn_beta"]
    g = inputs["Wg"][inputs["cluster_ids"]]
    b = inputs["Wb"][inputs["cluster_ids"]]
    ref = xn * (1.0 + g) + b
    err = np.max(np.abs(out - ref)) / np.max(np.abs(ref))
    print("rel err:", err)
